# revision 14
# baseline (speedup 1.0000x reference)
"""BinarizedLinear on 8 Trainium2 NeuronCores.

out = x @ sign(weight).T + bias
  x: (32768, 1024) f32, weight: (1024, 1024) f32, bias: (1024,) f32

Strategy (data-parallel over batch, weight/bias replicated):
  - each core handles a 4096-row shard of x
  - host marshals the shard feature-major (xT: [1024, 4096]) so every device
    DMA is large and contiguous; binarized weight is exact in bf16, shipped
    pre-transposed ([in, out]) so it can stream as the matmul moving operand
  - device: DMA x per 512KB feature-chunk -> DVE cast f32->bf16 -> PE matmul
    (x tile stationary, K=1024 accumulated in PSUM over 8 chunks) -> DVE
    bias-add (PSUM->SBUF) -> 512KB contiguous DMA out, natural layout
"""

import os
import sys

import numpy as np

sys.path.insert(0, "/opt/trn_rl_repo")

import ml_dtypes

import concourse.bass as bass
import concourse.tile as tile
from concourse import bacc, mybir
from concourse.bass_utils import run_bass_kernel_spmd

N_CORES = 8
B_FULL = 32768
I_DIM = 1024
O_DIM = 1024
BS = B_FULL // N_CORES  # 4096 batch rows per core

P = 128                # partitions / contraction tile
IC = I_DIM // P        # 8 contraction chunks
N_OC = 512             # psum free width (one PSUM bank of f32)
OC = O_DIM // N_OC     # 2 output chunks
B_SLAB = 1024          # batch columns per slab (4KB DRAM lines)
NSLAB = BS // B_SLAB   # 4 slabs
B_SUB = 128            # stationary-operand free width (psum partitions)
NSUB = B_SLAB // B_SUB # 8 subtiles per slab

# "bf16": one bf16 pass (x rounded to bf16; weight exact).
# "split": x = hi + lo bf16 decomposition, two accumulating passes -> ~fp32.
MODE = os.environ.get("BINLIN_MODE", "bf16")

F32 = mybir.dt.float32
BF16 = mybir.dt.bfloat16

_cache = {}


def _build_program(mode: str):
    nc = bacc.Bacc("TRN2", target_bir_lowering=False, debug=False,
                   num_devices=N_CORES)

    xt = nc.dram_tensor("xt", [I_DIM, BS], F32, kind="ExternalInput").ap()
    wt = nc.dram_tensor("wt", [I_DIM, O_DIM], BF16, kind="ExternalInput").ap()
    bias_d = nc.dram_tensor("bias_d", [1, O_DIM], F32,
                            kind="ExternalInput").ap()
    out = nc.dram_tensor("out", [BS, O_DIM], F32, kind="ExternalOutput").ap()

    n_parts = 2 if mode == "split" else 1

    HB = B_SLAB // 2  # x arrives in 256KB half-chunks for pipeline fill

    with tile.TileContext(nc) as tc:
        with (
            tc.tile_pool(name="consts", bufs=1) as consts,
            tc.tile_pool(name="xf", bufs=20) as xf_pool,
            tc.tile_pool(name="xb", bufs=5 * IC * n_parts // 2) as xb_pool,
            tc.tile_pool(name="ot", bufs=4) as ot_pool,
            tc.tile_pool(name="ps", bufs=4, space="PSUM") as ps_pool,
        ):
            # PE warmup: data-independent matmuls on scratch SBUF keep the
            # PE busy through DMA bring-up so HAM un-throttles to 2.4 GHz
            # before the first real matmul (results never read).
            warm_sc = consts.tile([P, N_OC], BF16)
            nc.gpsimd.memset(warm_sc[:], 0.0)
            ps_w = ps_pool.tile([P, N_OC], F32, tag="warm", bufs=1)
            for _ in range(10):
                nc.tensor.matmul(ps_w[:], warm_sc[:, :B_SUB], warm_sc[:],
                                 start=True, stop=True, skip_group_check=True)

            # Replicated constants on the scalar-engine HWDGE queue so they
            # don't delay the first x chunks on sync. Bias first (4KB HBM,
            # partition-broadcast by the DMA); then oc=0 weight columns --
            # the first psum groups only need that half of the weight.
            bias_sb = consts.tile([P, O_DIM], F32)
            nc.scalar.dma_start(bias_sb[:],
                                bias_d[0, :].partition_broadcast(P))
            wt_sb = consts.tile([P, IC * O_DIM], BF16)
            for oc in range(OC):
                for ic in range(IC):
                    nc.scalar.dma_start(
                        wt_sb[:, ic * O_DIM + oc * N_OC:
                              ic * O_DIM + oc * N_OC + N_OC],
                        wt[ic * P:(ic + 1) * P, oc * N_OC:(oc + 1) * N_OC])

            def load_chunks(sl):
                """Emit DMA+cast for one slab's x. Emitted one slab ahead of
                the matching compute so DVE casts sit ahead of the previous
                slab's PE-gated bias-adds in the DVE stream."""
                b0 = sl * B_SLAB
                xs_parts = [[[None, None] for _ in range(IC)]
                            for _ in range(n_parts)]
                for ic in range(IC):
                    for h in range(2):
                        hb0 = b0 + h * HB
                        xs_f = xf_pool.tile([P, HB], F32, tag="xs_f")
                        nc.sync.dma_start(
                            xs_f[:], xt[ic * P:(ic + 1) * P, hb0:hb0 + HB])
                        xs_hi = xb_pool.tile([P, HB], BF16, tag="xs_b")
                        nc.vector.tensor_copy(xs_hi[:], xs_f[:])
                        xs_parts[0][ic][h] = xs_hi
                        if mode == "split":
                            hi_f = xf_pool.tile([P, HB], F32, tag="hi_f")
                            nc.vector.tensor_copy(hi_f[:], xs_hi[:])
                            nc.vector.tensor_sub(hi_f[:], xs_f[:], hi_f[:])
                            xs_lo = xb_pool.tile([P, HB], BF16, tag="xs_b")
                            nc.vector.tensor_copy(xs_lo[:], hi_f[:])
                            xs_parts[1][ic][h] = xs_lo
                return xs_parts

            pending = load_chunks(0)
            for sl in range(NSLAB):
                b0 = sl * B_SLAB
                xs_parts = pending
                if sl + 1 < NSLAB:
                    pending = load_chunks(sl + 1)

                for su in range(NSUB):
                    h, c0 = divmod(su * B_SUB, HB)
                    last = sl == NSLAB - 1 and su == NSUB - 1
                    ot = ot_pool.tile([P, O_DIM], F32, tag="ot")
                    for oc in range(OC):
                        ps = ps_pool.tile([P, N_OC], F32, tag="ps")
                        n_mm = n_parts * IC
                        k = 0
                        for part in range(n_parts):
                            for ic in range(IC):
                                nc.tensor.matmul(
                                    ps[:],
                                    xs_parts[part][ic][h][:, c0:c0 + B_SUB],
                                    wt_sb[:, ic * O_DIM + oc * N_OC:
                                          ic * O_DIM + oc * N_OC + N_OC],
                                    start=(k == 0),
                                    stop=(k == n_mm - 1),
                                )
                                k += 1
                        nc.vector.tensor_add(
                            ot[:, oc * N_OC:(oc + 1) * N_OC], ps[:],
                            bias_sb[:, oc * N_OC:(oc + 1) * N_OC])
                        if last:
                            # tail: ship each half as soon as it's ready
                            r0 = b0 + su * B_SUB
                            nc.scalar.dma_start(
                                out[r0:r0 + B_SUB,
                                    oc * N_OC:(oc + 1) * N_OC],
                                ot[:, oc * N_OC:(oc + 1) * N_OC])
                    if not last:
                        r0 = b0 + su * B_SUB
                        # 512KB fully-contiguous store of 128 output rows.
                        nc.scalar.dma_start(out[r0:r0 + B_SUB, :], ot[:])

    nc.compile()
    return nc


def _get_program(mode: str):
    if mode not in _cache:
        _cache[mode] = _build_program(mode)
    return _cache[mode]


def _binarize_wt(weight: np.ndarray) -> np.ndarray:
    s = np.sign(weight)
    s[s == 0] = 1.0
    return np.ascontiguousarray(s.T).astype(ml_dtypes.bfloat16)


def kernel_impl(x, weight, bias, mode=MODE, trace=False, tmpdir=None):
    wt = _binarize_wt(np.asarray(weight))
    bias_d = np.ascontiguousarray(np.asarray(bias, np.float32)[None, :])
    x = np.asarray(x, np.float32)

    in_maps = []
    for c in range(N_CORES):
        xt = np.ascontiguousarray(x[c * BS:(c + 1) * BS].T)
        in_maps.append({"xt": xt, "wt": wt, "bias_d": bias_d})

    nc = _get_program(mode)
    res = run_bass_kernel_spmd(nc, in_maps, list(range(N_CORES)),
                               trace=trace, tmpdir=tmpdir)
    out = np.concatenate([res.results[c]["out"] for c in range(N_CORES)],
                         axis=0)
    return out, res


def kernel(x, weight, bias):
    out, _ = kernel_impl(x, weight, bias)
    return out


# revision 15
# speedup vs baseline: 1.0096x; 1.0096x over previous
"""BinarizedLinear on 8 Trainium2 NeuronCores.

out = x @ sign(weight).T + bias
  x: (32768, 1024) f32, weight: (1024, 1024) f32, bias: (1024,) f32

Strategy (data-parallel over batch, weight/bias replicated):
  - each core handles a 4096-row shard of x
  - host marshals the shard feature-major (xT: [1024, 4096]) so every device
    DMA is large and contiguous; binarized weight is exact in bf16, shipped
    pre-transposed ([in, out]) so it can stream as the matmul moving operand
  - device: DMA x per 512KB feature-chunk -> DVE cast f32->bf16 -> PE matmul
    (x tile stationary, K=1024 accumulated in PSUM over 8 chunks) -> DVE
    bias-add (PSUM->SBUF) -> 512KB contiguous DMA out, natural layout
"""

import os
import sys

import numpy as np

sys.path.insert(0, "/opt/trn_rl_repo")

import ml_dtypes

import concourse.bass as bass
import concourse.tile as tile
from concourse import bacc, mybir
from concourse.bass_utils import run_bass_kernel_spmd

N_CORES = 8
B_FULL = 32768
I_DIM = 1024
O_DIM = 1024
BS = B_FULL // N_CORES  # 4096 batch rows per core

P = 128                # partitions / contraction tile
IC = I_DIM // P        # 8 contraction chunks
N_OC = 512             # psum free width (one PSUM bank of f32)
OC = O_DIM // N_OC     # 2 output chunks
B_SLAB = 1024          # batch columns per slab (4KB DRAM lines)
NSLAB = BS // B_SLAB   # 4 slabs
B_SUB = 128            # stationary-operand free width (psum partitions)
NSUB = B_SLAB // B_SUB # 8 subtiles per slab

# "bf16": one bf16 pass (x rounded to bf16; weight exact).
# "split": x = hi + lo bf16 decomposition, two accumulating passes -> ~fp32.
MODE = os.environ.get("BINLIN_MODE", "bf16")

F32 = mybir.dt.float32
BF16 = mybir.dt.bfloat16

_cache = {}


def _build_program(mode: str):
    nc = bacc.Bacc("TRN2", target_bir_lowering=False, debug=False,
                   num_devices=N_CORES)

    xt = nc.dram_tensor("xt", [I_DIM, BS], F32, kind="ExternalInput").ap()
    wt = nc.dram_tensor("wt", [I_DIM, O_DIM], mybir.dt.float8e4,
                        kind="ExternalInput").ap()
    bias_d = nc.dram_tensor("bias_d", [1, O_DIM], F32,
                            kind="ExternalInput").ap()
    out = nc.dram_tensor("out", [BS, O_DIM], F32, kind="ExternalOutput").ap()

    n_parts = 2 if mode == "split" else 1

    HB = B_SLAB // 2  # x arrives in 256KB half-chunks for pipeline fill

    with tile.TileContext(nc) as tc:
        with (
            tc.tile_pool(name="consts", bufs=1) as consts,
            tc.tile_pool(name="xf", bufs=36) as xf_pool,
            tc.tile_pool(name="xb", bufs=5 * IC * n_parts) as xb_pool,
            tc.tile_pool(name="ot", bufs=4) as ot_pool,
            tc.tile_pool(name="ps", bufs=4, space="PSUM") as ps_pool,
        ):
            # PE warmup: data-independent matmuls on scratch SBUF keep the
            # PE busy through DMA bring-up so HAM un-throttles to 2.4 GHz
            # before the first real matmul (results never read).
            warm_sc = consts.tile([P, N_OC], BF16)
            nc.gpsimd.memset(warm_sc[:], 0.0)
            ps_w = ps_pool.tile([P, N_OC], F32, tag="warm", bufs=1)
            for _ in range(10):
                nc.tensor.matmul(ps_w[:], warm_sc[:, :B_SUB], warm_sc[:],
                                 start=True, stop=True, skip_group_check=True)

            # Replicated constants on the scalar-engine HWDGE queue so they
            # don't delay the first x chunks on sync. Bias first (4KB HBM,
            # partition-broadcast by the DMA); then oc=0 weight columns --
            # the first psum groups only need that half of the weight.
            bias_sb = consts.tile([P, O_DIM], F32)
            nc.scalar.dma_start(bias_sb[:],
                                bias_d[0, :].partition_broadcast(P))
            wt_sb = consts.tile([P, IC * O_DIM], mybir.dt.float8e4)
            for oc in range(OC):
                for ic in range(IC):
                    nc.scalar.dma_start(
                        wt_sb[:, ic * O_DIM + oc * N_OC:
                              ic * O_DIM + oc * N_OC + N_OC],
                        wt[ic * P:(ic + 1) * P, oc * N_OC:(oc + 1) * N_OC])

            def load_chunks(sl):
                """Emit DMA+cast for one slab's x. Emitted one slab ahead of
                the matching compute so DVE casts sit ahead of the previous
                slab's PE-gated bias-adds in the DVE stream."""
                b0 = sl * B_SLAB
                xs_parts = [[[None, None] for _ in range(IC)]
                            for _ in range(n_parts)]
                for ic in range(IC):
                    for h in range(2):
                        hb0 = b0 + h * HB
                        xs_f = xf_pool.tile([P, HB], F32, tag="xs_f")
                        nc.sync.dma_start(
                            xs_f[:], xt[ic * P:(ic + 1) * P, hb0:hb0 + HB])
                        xs_hi = xb_pool.tile([P, HB], BF16, tag="xs_b")
                        nc.vector.tensor_copy(xs_hi[:], xs_f[:])
                        xs_parts[0][ic][h] = xs_hi
                        if mode == "split":
                            hi_f = xf_pool.tile([P, HB], F32, tag="hi_f")
                            nc.vector.tensor_copy(hi_f[:], xs_hi[:])
                            nc.vector.tensor_sub(hi_f[:], xs_f[:], hi_f[:])
                            xs_lo = xb_pool.tile([P, HB], BF16, tag="xs_b")
                            nc.vector.tensor_copy(xs_lo[:], hi_f[:])
                            xs_parts[1][ic][h] = xs_lo
                return xs_parts

            pending = [load_chunks(0), load_chunks(1)]
            for sl in range(NSLAB):
                b0 = sl * B_SLAB
                xs_parts = pending.pop(0)
                if sl + 2 < NSLAB:
                    pending.append(load_chunks(sl + 2))

                for su in range(NSUB):
                    h, c0 = divmod(su * B_SUB, HB)
                    last = sl == NSLAB - 1 and su == NSUB - 1
                    ot = ot_pool.tile([P, O_DIM], F32, tag="ot")
                    for oc in range(OC):
                        ps = ps_pool.tile([P, N_OC], F32, tag="ps")
                        n_mm = n_parts * IC
                        k = 0
                        for part in range(n_parts):
                            for ic in range(IC):
                                nc.tensor.matmul(
                                    ps[:],
                                    xs_parts[part][ic][h][:, c0:c0 + B_SUB],
                                    wt_sb[:, ic * O_DIM + oc * N_OC:
                                          ic * O_DIM + oc * N_OC + N_OC],
                                    start=(k == 0),
                                    stop=(k == n_mm - 1),
                                )
                                k += 1
                        nc.vector.tensor_add(
                            ot[:, oc * N_OC:(oc + 1) * N_OC], ps[:],
                            bias_sb[:, oc * N_OC:(oc + 1) * N_OC])
                        if last:
                            # tail: ship each half as soon as it's ready
                            r0 = b0 + su * B_SUB
                            nc.scalar.dma_start(
                                out[r0:r0 + B_SUB,
                                    oc * N_OC:(oc + 1) * N_OC],
                                ot[:, oc * N_OC:(oc + 1) * N_OC])
                    if not last:
                        r0 = b0 + su * B_SUB
                        # 512KB fully-contiguous store of 128 output rows.
                        nc.scalar.dma_start(out[r0:r0 + B_SUB, :], ot[:])

    nc.compile()
    return nc


def _get_program(mode: str):
    if mode not in _cache:
        _cache[mode] = _build_program(mode)
    return _cache[mode]


def _binarize_wt(weight: np.ndarray) -> np.ndarray:
    s = np.sign(weight)
    s[s == 0] = 1.0
    return np.ascontiguousarray(s.T).astype(ml_dtypes.float8_e4m3)


def kernel_impl(x, weight, bias, mode=MODE, trace=False, tmpdir=None):
    wt = _binarize_wt(np.asarray(weight))
    bias_d = np.ascontiguousarray(np.asarray(bias, np.float32)[None, :])
    x = np.asarray(x, np.float32)

    in_maps = []
    for c in range(N_CORES):
        xt = np.ascontiguousarray(x[c * BS:(c + 1) * BS].T)
        in_maps.append({"xt": xt, "wt": wt, "bias_d": bias_d})

    nc = _get_program(mode)
    res = run_bass_kernel_spmd(nc, in_maps, list(range(N_CORES)),
                               trace=trace, tmpdir=tmpdir)
    out = np.concatenate([res.results[c]["out"] for c in range(N_CORES)],
                         axis=0)
    return out, res


def kernel(x, weight, bias):
    out, _ = kernel_impl(x, weight, bias)
    return out
